# revision 8
# baseline (speedup 1.0000x reference)
"""LIF spiking-neuron (soft reset) Bass kernel for Trainium2, 8-core SPMD.

Input  x: [B=32, C=128, T=16, H=32, W=32] f32
Output s: same shape, spikes in {0, 1}.

Bit-exact recurrence per element over T (matches the jax reference):
    m' = m * 0.75 + x_t      (round: mul, then add)
    s  = (m' > 0.5)
    m  = m' - 0.5 * s        (single-rounding subtract)

Sharding: batch dim split across 8 cores (4 per core); rows = b*c mapped to
SBUF partitions in 4 groups of 128, HW=1024 on the free axis.

Engine plan (from NTFF traces / microbenches, per [128,1024] op):
  - GpSimd (Pool) elementwise work poisons concurrently-dispatched DVE ops
    ~2.8x (shared SBUF port), so GpSimd does nothing but one weight DMA.
  - threshold u_j = (m' > 0.5) * 2^j in bf16: DVE tensor_scalar, pair-fused
    [128,2048] (~1.22 us/pair, 2x mode). The 2^j scaling lets every pack
    matmul share a single identity bf16 weight (one LDWEIGHTS per burst).
  - subtract d = m' - 0.5*s, scheduled per (t,g) over three exact variants:
      "dve":   stt(u_j * -2^-(j+1) + m') on DVE (~1.22 us)
      "pe":    I_f32 @ m' then -2^-(j+1)*I_bf16 @ u_j into PSUM (~2.4 us PE)
      "actpe": Act copies m' into PSUM (~1.15 us), PE accumulates the bf16
               -2^-(j+1)*I @ u_j on top (~0.8 us PE)
    All bit-exact: products by powers of two are exact, the PSUM/ALU add is
    the same single IEEE f32 rounding the reference performs.
  - m-update: one DVE stt m = (d * 0.75) + x_next; the stt rounds the f32
    intermediate product, matching the reference mul-then-add exactly.
  - output bit-pack: PSUM pack = sum_j I_bf16 @ u_j = sum_j 2^j s_j, copied
    to uint8 by Act; 2 bytes per 16-step chain instead of 16 f32 words
    (32x less store traffic). Host unpacks bits to f32 (exact).
"""

import numpy as np

import concourse.bacc as bacc
import concourse.mybir as mybir
import concourse.tile as tile
from concourse.bass_utils import run_bass_kernel_spmd

B, C, T, H, W = 32, 128, 16, 32, 32
NCORES = 8
B_PER = B // NCORES          # 4
ROWS = B_PER * C             # 512
HW = H * W                   # 1024
P = 128
NG = ROWS // P               # 4
NQ = NG // 2                 # 2 group-pairs
HALF = HW // 2               # 512 = psum bank / fp32 matmul free-dim limit
BETA = 0.75
THRESH = 0.5

F32 = mybir.dt.float32
BF16 = mybir.dt.bfloat16
U8 = mybir.dt.uint8
ALU = mybir.AluOpType

# sub engine per (t, g), cycling over idx = t*NG + g
SUB_CYCLE = ["pe", "pe", "dve", "pe", "pe", "dve"]

_nc_cache = None


def _build():
    nc = bacc.Bacc(
        "TRN2",
        target_bir_lowering=False,
        debug=False,
        enable_asserts=False,
        num_devices=NCORES,
    )
    x_d = nc.dram_tensor("x", [ROWS, T, HW], F32, kind="ExternalInput").ap()
    p_d = nc.dram_tensor("s", [ROWS, 2, HW], U8, kind="ExternalOutput").ap()

    x_v = x_d.rearrange("(g p) t f -> g p t f", p=P)
    p_v = p_d.rearrange("(g p) k f -> g p k f", p=P)

    eye = np.eye(P, dtype=np.float32)
    # bf16 weights: slots 0..7 = -2^-(j+1)*I (subtract), slot 8 = I (pack)
    wall_np = np.zeros((P, 9 * P), dtype=np.float32)
    for j in range(8):
        wall_np[:, j * P:(j + 1) * P] = eye * (-(2.0 ** -(j + 1)))
    wall_np[:, 8 * P:] = eye
    wall_d = nc.inline_tensor(wall_np, name="wall")
    wi_d = nc.inline_tensor(eye, name="wi")

    with tile.TileContext(nc) as tc:
        with (
            tc.tile_pool(name="mp", bufs=1) as m_pool,
            tc.tile_pool(name="xp", bufs=3) as x_pool,
            tc.tile_pool(name="up", bufs=1) as u_pool,
            tc.tile_pool(name="dp", bufs=4) as d_pool,
            tc.tile_pool(name="op", bufs=2) as o_pool,
            tc.tile_pool(name="wp", bufs=1) as w_pool,
            tc.tile_pool(name="pkp", bufs=2, space="PSUM") as pk_pool,
            tc.tile_pool(name="dsp", bufs=2, space="PSUM") as ds_pool,
        ):
            wall = w_pool.tile([P, 9 * P], BF16, tag="wall", name="wall")
            nc.gpsimd.dma_start(wall[:], wall_d.ap()[:])  # f32 -> bf16 cast
            wi = w_pool.tile([P, P], F32, tag="wi", name="wi")
            nc.sync.dma_start(wi[:], wi_d.ap()[:])

            def w_sub(j):
                return wall[:, j * P:(j + 1) * P]

            w_pk = wall[:, 8 * P:]

            def load_x(t, split=False):
                xt = x_pool.tile([P, NG, HW], F32, tag="xt", name="xt")
                if split:
                    for g in range(NG):
                        nc.sync.dma_start(xt[:, g, :], x_v[g, :, t, :])
                else:
                    nc.sync.dma_start(
                        xt[:], x_v[:, :, t, :].rearrange("g p f -> p g f")
                    )
                return xt

            m_tiles = [
                m_pool.tile([P, 2, HW], F32, tag=f"m{q}", name=f"m{q}")
                for q in range(NQ)
            ]
            u_tiles = [
                [
                    u_pool.tile([P, 2, HW], BF16, tag=f"u{q}_{j}", name=f"u{q}_{j}")
                    for j in range(8)
                ]
                for q in range(NQ)
            ]

            x0 = load_x(0, split=True)
            xs = {0: x0}

            for t in range(T):
                j = t % 8
                k = t // 8
                pj = float(2.0 ** j)
                cj = -(2.0 ** -(j + 1))
                pair_srcs = [
                    x0[:, 2 * q:2 * q + 2, :] if t == 0 else m_tiles[q][:]
                    for q in range(NQ)
                ]
                if t < T - 1:
                    xs[t + 1] = load_x(t + 1)

                # thresholds, pair-fused on DVE: u_j = (m' > 0.5) * 2^j (bf16)
                for q in range(NQ):
                    nc.vector.tensor_scalar(
                        u_tiles[q][j][:], pair_srcs[q], THRESH, pj,
                        ALU.is_gt, ALU.mult,
                    )

                if t < T - 1:
                    x_next = xs[t + 1]
                    ds = []
                    for g in range(NG):
                        q, i = divmod(g, 2)
                        src_g = pair_srcs[q][:, i, :]
                        u_g = u_tiles[q][j][:, i, :]
                        mode = SUB_CYCLE[(t * NG + g) % len(SUB_CYCLE)]
                        if mode == "pe":
                            d = ds_pool.tile([P, HW], F32, tag="dps", name="dps")
                            for c in range(2):
                                cols = slice(c * HALF, (c + 1) * HALF)
                                nc.tensor.matmul(
                                    d[:, cols], wi[:], src_g[:, cols],
                                    start=True, stop=False,
                                )
                                nc.tensor.matmul(
                                    d[:, cols], w_sub(j), u_g[:, cols],
                                    start=False, stop=True,
                                )
                        elif mode == "actpe":
                            d = ds_pool.tile([P, HW], F32, tag="dps", name="dps")
                            nc.scalar.copy(d[:], src_g)
                            for c in range(2):
                                cols = slice(c * HALF, (c + 1) * HALF)
                                nc.tensor.matmul(
                                    d[:, cols], w_sub(j), u_g[:, cols],
                                    start=False, stop=True,
                                    skip_group_check=True,
                                )
                        else:
                            d = d_pool.tile([P, HW], F32, tag="dt", name="dt")
                            nc.vector.scalar_tensor_tensor(
                                d[:], u_g, cj, src_g,
                                op0=ALU.mult, op1=ALU.add,
                            )
                        ds.append(d)
                    # m-update: m = (d * 0.75) + x_next, one DVE stt
                    for g in range(NG):
                        q, i = divmod(g, 2)
                        nc.vector.scalar_tensor_tensor(
                            m_tiles[q][:, i, :], ds[g][:], BETA,
                            x_next[:, g, :], op0=ALU.mult, op1=ALU.add,
                        )

                if j == 7:
                    out_sb = o_pool.tile([P, NG, HW], U8, tag="st", name="st")
                    for g in range(NG):
                        q, i = divmod(g, 2)
                        pk = pk_pool.tile([P, HW], F32, tag="pk", name="pk")
                        for jj in range(8):
                            for c in range(2):
                                cols = slice(c * HALF, (c + 1) * HALF)
                                nc.tensor.matmul(
                                    pk[:, cols], w_pk,
                                    u_tiles[q][jj][:, i, cols],
                                    start=(jj == 0), stop=(jj == 7),
                                )
                        nc.scalar.copy(out_sb[:, g, :], pk[:])
                    nc.sync.dma_start(
                        p_v[:, :, k, :].rearrange("g p f -> p g f"), out_sb[:]
                    )
    nc.compile()
    return nc


def _get_nc():
    global _nc_cache
    if _nc_cache is None:
        _nc_cache = _build()
    return _nc_cache


def _run(x, **spmd_kwargs):
    x = np.ascontiguousarray(np.asarray(x, dtype=np.float32))
    assert x.shape == (B, C, T, H, W)
    nc = _get_nc()
    in_maps = [
        {"x": x[i * B_PER:(i + 1) * B_PER].reshape(ROWS, T, HW)}
        for i in range(NCORES)
    ]
    res = run_bass_kernel_spmd(nc, in_maps, list(range(NCORES)), **spmd_kwargs)
    outs = []
    for r in res.results:
        packed = r["s"]  # [ROWS, 2, HW] u8
        bits = np.unpackbits(packed, axis=1, bitorder="little")  # [ROWS,16,HW]
        outs.append(bits.reshape(B_PER, C, T, H, W).astype(np.float32))
    return np.concatenate(outs, axis=0), res


def kernel(x):
    out, _ = _run(x)
    return out
